# revision 53
# baseline (speedup 1.0000x reference)
"""RGCN message-scoring kernel for Trainium2 (8 NeuronCores, SPMD).

Strategy (sharding_hint: partition graphs across devices):
- 250 graphs of exactly 200 nodes / 3200 within-graph edges are split across
  8 cores ([32,32,31,...,31], padded with zero "dummy" graphs to 32 each).
- Host does index-only preprocessing: per-graph normalized adjacency
  operator B[src, (rel,dst)] (counts -> 1/cnt edge weights, bf16) stored
  pre-split into a 128-row chunk and a 72-row chunk so the device needs no
  zero padding; plus layout rearrangement of the dense inputs. All FP model
  math (both RGCN layers, message MLP, scoring) runs on device.
- Device (per graph): T1 = x^T @ B (PE), staged relation-major per graph
  PAIR so the layer-1 transform runs 9 matmuls per pair (free dim 400),
  ReLU (ACT); layer 2 is collapsed against the per-graph message vector:
  psi = h1 @ [W2_r m | root2 m] and scores = sum_r psi_r^T B_r, so layer 2
  never materializes node embeddings.
- B arrives in blocks of 4 graphs (2 large DMAs per block, double
  buffered), so DMA is fully overlapped with PE work.
- Output [250, 200] is a per-graph reshape of the per-node scores (each
  graph has exactly max_nodes nodes, so no -inf padding survives).
"""

import numpy as np

NG_FULL = 250       # total graphs
NPG = 200           # nodes per graph
EPG = 3200          # edges per graph
R = 8               # relations
F = 128             # feature/embedding width
G = 32              # graphs per core (padded)
NCORES = 8
KEYS = R * NPG      # 1600, relation-major: k = r*200 + dst_local
BL = 2              # graphs per DMA block
C1 = 72             # rows in second node chunk (200 - 128)

_COMPILED = {}


def _bf16(a):
    import ml_dtypes
    return np.ascontiguousarray(np.asarray(a, np.float32).astype(ml_dtypes.bfloat16))


def _emit(nc, tc, io, ablate=None):
    """Emit the full per-core program body (input loads + compute + store).

    Called once for the production program; the timing harness may wrap it
    in a repeat loop to measure steady-state per-execution time. `ablate`
    ("dma_only" | "t1_only" | "no_sc") builds reduced timing-only variants
    for bottleneck attribution.
    """
    import concourse.mybir as mybir
    dt = mybir.dt
    AF = mybir.ActivationFunctionType
    T = io.__getitem__

    NPAIR = G // 2
    PAIRS_PER_BLK = BL // 2
    NBLK = G // BL
    with (
        tc.tile_pool(name="const", bufs=1) as const,
        tc.tile_pool(name="bpool", bufs=3) as bpool,
        tc.tile_pool(name="xlpool", bufs=1) as xlpool,
        tc.tile_pool(name="t1pool", bufs=2) as t1pool,
        tc.tile_pool(name="hpool", bufs=2) as hpool,
    ):
        # xl split per block so the first T1 only waits on its own slice
        xl_blk = []
        for b in range(NBLK):
            xlb = xlpool.tile([128, BL * 2 * 128], dt.bfloat16,
                              tag=f"xl{b}", name=f"xl{b}")
            xl_blk.append(xlb)
        bc_of = {}

        def load_block(blk):
            bc0 = bpool.tile([128, BL * KEYS], dt.bfloat16, tag="bc0")
            bc1 = bpool.tile([C1, BL * KEYS], dt.bfloat16, tag="bc1")
            nc.sync.dma_start(
                bc0[:], T("BC0").ap()[:, blk * BL * KEYS:(blk + 1) * BL * KEYS])
            nc.sync.dma_start(
                bc1[:], T("BC1").ap()[:, blk * BL * KEYS:(blk + 1) * BL * KEYS])
            nc.sync.dma_start(
                xl_blk[blk][:],
                T("XL").ap()[:, blk * BL * 2 * 128:(blk + 1) * BL * 2 * 128])
            bc_of[blk] = (bc0, bc1)

        # critical-path-first DMA order: block 0 (B operator + x slice),
        # then the small message-stage inputs, then remaining constants.
        load_block(0)

        w2mB = const.tile([128, 9 * G], dt.bfloat16)
        b2m = const.tile([1, G], dt.float32)
        out_sb = const.tile([1, G * NPG], dt.float32)
        w2t = const.tile([128, R * 128], dt.bfloat16)
        root2t = const.tile([128, 128], dt.bfloat16)
        b2 = const.tile([128, 1], dt.bfloat16)
        nc.sync.dma_start(w2t[:], T("W2T").ap()[:])
        nc.sync.dma_start(root2t[:], T("ROOT2T").ap()[:])
        nc.sync.dma_start(b2[:], T("B2").ap()[:])
        if ablate in ("dma_only", "t1_only"):
            nc.vector.memset(w2mB[:], 0.0)
            nc.vector.memset(b2m[:], 0.0)
            nc.vector.memset(out_sb[:], 0.0)
        if ablate is None or ablate == "no_sc":
            _emit_message(nc, tc, io, w2mB, b2m, w2t, root2t, b2)
            if ablate == "no_sc":
                nc.vector.memset(out_sb[:], 0.0)

        xt = const.tile([128, G * NPG], dt.bfloat16)
        w1l = const.tile([128, R * 128], dt.bfloat16)
        root1 = const.tile([128, 128], dt.bfloat16)
        b1 = const.tile([128, 1], dt.float32)
        nc.sync.dma_start(xt[:], T("XT").ap()[:])
        nc.sync.dma_start(w1l[:], T("W1L").ap()[:])
        nc.sync.dma_start(root1[:], T("ROOT1").ap()[:])
        nc.sync.dma_start(b1[:], T("B1").ap()[:])

        # ---- per-graph pipeline, software-pipelined by graph PAIR:
        # emission order per step p: a1(p-1) [PE], T1(p) [PE] (fills PE
        # while ACT runs relu(p-1) and DVE drains psi), then psi+scores
        # (p-1). PSUM banks: 3 (T1) + 1 (a1) + 2 (psi) + 2 (scores) = 8.
        w2mB3 = w2mB[:].rearrange("p (n g) -> p n g", g=G)
        with (
            tc.tile_pool(name="pst1", bufs=3, space="PSUM") as pst1,
            tc.tile_pool(name="psa", bufs=1, space="PSUM") as psa,
            tc.tile_pool(name="psp", bufs=2, space="PSUM") as psp,
            tc.tile_pool(name="pss", bufs=2, space="PSUM") as pss,
        ):
            def emit_t1(p):
                blk = p // PAIRS_PER_BLK
                bc0, bc1 = bc_of[blk]
                xlb = xl_blk[blk]
                pair = p % PAIRS_PER_BLK
                # t1sb: relation-major pair staging, col = r*400 + pi*200 + k
                t1sb = t1pool.tile([128, R * 2 * NPG], dt.bfloat16)
                t1v = t1sb[:].rearrange("p (r t k) -> p r t k", r=R, t=2)
                for pi in range(2):
                    gi = pair * 2 + pi               # graph within block
                    # T1 [128f, 1600keys] in 4 psum tiles of 400 cols
                    for q in range(4):
                        t1p = pst1.tile([128, 400], dt.float32)
                        base = gi * KEYS + q * 400
                        nc.tensor.matmul(
                            t1p[:],
                            xlb[:, (gi * 2) * 128:(gi * 2 + 1) * 128],
                            bc0[:, base:base + 400],
                            start=True, stop=False)
                        nc.tensor.matmul(
                            t1p[:],
                            xlb[0:C1, (gi * 2 + 1) * 128:(gi * 2 + 2) * 128],
                            bc1[:, base:base + 400],
                            start=False, stop=True)
                        # scatter the 2 relations of this quarter into t1sb;
                        # alternate DVE/ACT so the psum drain keeps pace with
                        # the Tensor engine's 2-matmul fill rate
                        dst = t1v[:, q * 2:q * 2 + 2, pi, :]
                        src = t1p[:].rearrange("p (q k) -> p q k", q=2)
                        if q % 2 == 0:
                            nc.vector.tensor_copy(dst, src)
                        else:
                            nc.scalar.activation(dst, src, AF.Copy)
                return t1sb

            def emit_a1(p, t1sb):
                g0 = p * 2                            # first graph of pair
                # a1 [128emb, 400] = sum_r W1_r^T T1_r + root1^T x^T
                a1 = psa.tile([128, 2 * NPG], dt.float32)
                for r in range(R):
                    nc.tensor.matmul(a1[:], w1l[:, r * 128:(r + 1) * 128],
                                     t1sb[:, r * 400:(r + 1) * 400],
                                     start=(r == 0), stop=False)
                nc.tensor.matmul(a1[:], root1[:],
                                 xt[:, g0 * NPG:(g0 + 2) * NPG],
                                 start=False, stop=True)
                h1 = hpool.tile([128, 2 * NPG], dt.bfloat16)
                nc.scalar.activation(h1[:], a1[:], AF.Relu, bias=b1[:])
                if ablate == "no_sc":
                    nc.scalar.activation(out_sb[0:1, p * 400:(p + 1) * 400],
                                         h1[0:1, :], AF.Copy)
                return h1

            def emit_score(p, h1):
                blk = p // PAIRS_PER_BLK
                bc0, bc1 = bc_of[blk]
                pair = p % PAIRS_PER_BLK
                # psi [node, 9] per node-chunk, both graphs first (so PE can
                # run graph 1's psi while DVE drains graph 0's)
                psis = []
                for pi in range(2):
                    g = blk * BL + pair * 2 + pi
                    hoff = pi * NPG
                    psi_p = psp.tile([128, 18], dt.float32)
                    nc.tensor.matmul(psi_p[:, 0:9], h1[:, hoff:hoff + 128],
                                     w2mB3[:, :, g], start=True, stop=True)
                    nc.tensor.matmul(psi_p[:C1, 9:18],
                                     h1[:, hoff + 128:hoff + NPG],
                                     w2mB3[:, :, g], start=True, stop=True)
                    psi = hpool.tile([128, 18], dt.bfloat16, tag="psi")
                    nc.vector.tensor_copy(psi[:, 0:9], psi_p[:, 0:9])
                    nc.vector.tensor_copy(psi[:C1, 9:18], psi_p[:C1, 9:18])
                    psis.append(psi)

                for pi in range(2):
                    gi = pair * 2 + pi
                    g = blk * BL + gi
                    psi = psis[pi]
                    # scores [1, 200] = sum_r psi_r^T B_r + (root2 m)^T h1
                    sc = pss.tile([1, NPG], dt.float32)
                    for r in range(R):
                        nc.tensor.matmul(
                            sc[:], psi[:, r:r + 1],
                            bc0[:, gi * KEYS + r * NPG:gi * KEYS + (r + 1) * NPG],
                            start=(r == 0), stop=False)
                    for r in range(R):
                        nc.tensor.matmul(
                            sc[:], psi[0:C1, 9 + r:10 + r],
                            bc1[:, gi * KEYS + r * NPG:gi * KEYS + (r + 1) * NPG],
                            start=False, stop=False)
                    nc.tensor.matmul(sc[:], w2mB[:, 8 * G + g:8 * G + g + 1],
                                     h1[:, pi * NPG:(pi + 1) * NPG],
                                     start=False, stop=True)
                    nc.scalar.activation(out_sb[0:1, g * NPG:(g + 1) * NPG],
                                         sc[:], AF.Identity,
                                         bias=b2m[0:1, g:g + 1])

            if ablate == "dma_only":
                stride = G * NPG // NBLK
                for blk in range(1, NBLK):
                    load_block(blk)
                for blk in range(NBLK):
                    bc0, bc1 = bc_of[blk]
                    o = blk * stride
                    nc.scalar.activation(out_sb[0:1, o:o + stride // 2],
                                         bc0[0:1, 0:stride // 2], AF.Copy)
                    nc.scalar.activation(out_sb[0:1, o + stride // 2:o + stride],
                                         bc1[0:1, 0:stride // 2], AF.Copy)
            else:
                next_blk = 1
                prev = None
                for p in range(NPAIR):
                    cur_blk = p // PAIRS_PER_BLK
                    while next_blk < min(NBLK, cur_blk + 3):
                        load_block(next_blk)
                        next_blk += 1
                    if prev is not None:
                        h1 = emit_a1(p - 1, prev)
                    t1sb = emit_t1(p)
                    if ablate == "t1_only":
                        nc.scalar.activation(out_sb[0:1, p * 400:(p + 1) * 400],
                                             t1sb[0:1, 0:400], AF.Copy)
                        prev = None
                        continue
                    if prev is not None and ablate != "no_sc":
                        emit_score(p - 1, h1)
                    prev = t1sb
                if prev is not None:
                    h1 = emit_a1(NPAIR - 1, prev)
                    if ablate != "no_sc":
                        emit_score(NPAIR - 1, h1)

        nc.sync.dma_start(T("OUT").ap()[:], out_sb[:])


def _emit_message(nc, tc, io, w2mB, b2m, w2t, root2t, b2):
    """Message MLP + collapse of layer-2 weights against each graph's
    message vector: w2mB[:, n*G+g] = W2_n m_g (n<8) / root2 m_g (n==8),
    b2m[g] = b2 . m_g."""
    import concourse.mybir as mybir
    dt = mybir.dt
    AF = mybir.ActivationFunctionType
    T = io.__getitem__

    with (
        tc.tile_pool(name="msg", bufs=1) as msg,
        tc.tile_pool(name="psm", bufs=2, space="PSUM") as psm,
    ):
        embl = msg.tile([128, 8 * 128], dt.bfloat16)
        sel = msg.tile([128, 8 * G], dt.bfloat16)
        cont = msg.tile([1, G], dt.bfloat16)
        contw = msg.tile([1, 128], dt.bfloat16)
        contb = msg.tile([128, 1], dt.float32)
        msgw = msg.tile([128, 2 * 128], dt.bfloat16)
        msgb = msg.tile([128, 1], dt.float32)
        nc.sync.dma_start(embl[:], T("EMBL").ap()[:])
        nc.sync.dma_start(sel[:], T("SEL").ap()[:])
        nc.sync.dma_start(cont[:], T("CONT").ap()[:])
        nc.sync.dma_start(contw[:], T("CONTW").ap()[:])
        nc.sync.dma_start(contb[:], T("CONTB").ap()[:])
        nc.sync.dma_start(msgw[:], T("MSGW").ap()[:])
        nc.sync.dma_start(msgb[:], T("MSGB").ap()[:])

        # disc_embT [128f, G] = sum_c EMBL_c^T @ SEL_c
        ps_d = psm.tile([128, G], dt.float32)
        for c in range(8):
            nc.tensor.matmul(
                ps_d[:], embl[:, c * 128:(c + 1) * 128], sel[:, c * G:(c + 1) * G],
                start=(c == 0), stop=(c == 7))
        discT = msg.tile([128, G], dt.bfloat16)
        nc.vector.tensor_copy(discT[:], ps_d[:])

        # cont_embT [128, G] = relu(cont_w^T cont + cont_b)
        ps_c = psm.tile([128, G], dt.float32)
        nc.tensor.matmul(ps_c[:], contw[:], cont[:], start=True, stop=True)
        contT = msg.tile([128, G], dt.bfloat16)
        nc.scalar.activation(contT[:], ps_c[:], AF.Relu, bias=contb[:])

        # mT [128, G] = relu(msg_w^T [disc; cont] + msg_b)
        ps_m = psm.tile([128, G], dt.float32)
        nc.tensor.matmul(ps_m[:], msgw[:, 0:128], discT[:], start=True, stop=False)
        nc.tensor.matmul(ps_m[:], msgw[:, 128:256], contT[:], start=False, stop=True)
        mT = msg.tile([128, G], dt.bfloat16)
        nc.scalar.activation(mT[:], ps_m[:], AF.Relu, bias=msgb[:])

        # w2m[r] = W2_r m ; rootm = root2 m ; b2m = b2 . m
        for r in range(R):
            ps_w = psm.tile([128, G], dt.float32, tag="psw")
            nc.tensor.matmul(ps_w[:], w2t[:, r * 128:(r + 1) * 128], mT[:],
                             start=True, stop=True)
            nc.vector.tensor_copy(w2mB[:, r * G:(r + 1) * G], ps_w[:])
        ps_r = psm.tile([128, G], dt.float32, tag="psw")
        nc.tensor.matmul(ps_r[:], root2t[:], mT[:], start=True, stop=True)
        nc.vector.tensor_copy(w2mB[:, 8 * G:9 * G], ps_r[:])
        ps_b = psm.tile([1, G], dt.float32, tag="psw")
        nc.tensor.matmul(ps_b[:], b2[:], mT[:], start=True, stop=True)
        nc.vector.tensor_copy(b2m[:], ps_b[:])


def _declare_io(nc):
    import concourse.mybir as mybir
    dt = mybir.dt
    io = {}
    specs = [
        ("XL", [128, G * 2 * 128], dt.bfloat16),
        ("XT", [128, G * NPG], dt.bfloat16),
        ("BC0", [128, G * KEYS], dt.bfloat16),
        ("BC1", [C1, G * KEYS], dt.bfloat16),
        ("W1L", [128, R * 128], dt.bfloat16),
        ("ROOT1", [128, 128], dt.bfloat16),
        ("B1", [128, 1], dt.float32),
        ("W2T", [128, R * 128], dt.bfloat16),
        ("ROOT2T", [128, 128], dt.bfloat16),
        ("B2", [128, 1], dt.bfloat16),
        ("EMBL", [128, 8 * 128], dt.bfloat16),
        ("SEL", [128, 8 * G], dt.bfloat16),
        ("CONT", [1, G], dt.bfloat16),
        ("CONTW", [1, 128], dt.bfloat16),
        ("CONTB", [128, 1], dt.float32),
        ("MSGW", [128, 2 * 128], dt.bfloat16),
        ("MSGB", [128, 1], dt.float32),
    ]
    for name, shape, dtype in specs:
        io[name] = nc.dram_tensor(name, shape, dtype, kind="ExternalInput")
    io["OUT"] = nc.dram_tensor("OUT", [1, G * NPG], dt.float32,
                               kind="ExternalOutput")
    return io


def _build_program(loop_k=None, ablate=None, unroll=1):
    """Build the per-core program. With loop_k, the full body (including all
    input DMA) repeats loop_k*unroll times per NEFF execution — used by the
    timing harness to measure steady-state per-execution time. unroll>1
    places consecutive bodies in one loop iteration so the scheduler can
    overlap one execution's output tail with the next one's input ramp
    (the For_i back-edge is an all-engine barrier)."""
    import concourse.bacc as bacc
    from concourse import tile

    import concourse.mybir as mybir

    nc = bacc.Bacc("TRN2", target_bir_lowering=False, debug=False)
    io = _declare_io(nc)
    with tile.TileContext(nc) as tc:
        if loop_k is None:
            _emit(nc, tc, io, ablate=ablate)
        else:
            with tc.For_i(0, loop_k, 1,
                          hint_engines=(mybir.EngineType.PE,)):
                for _ in range(unroll):
                    _emit(nc, tc, io, ablate=ablate)
    nc.compile()
    return nc


def _np_reference(message, x, edge_index, edge_type, batch, max_nodes,
                  W1, root1, b1, W2, root2, b2,
                  embed_table, cont_w, cont_b, msg_w, msg_b):
    """Pure-numpy fallback for inputs that violate the regular-structure
    assumptions (ragged batches or cross-graph edges)."""
    n_nodes, n_rel, n_graphs = x.shape[0], W1.shape[0], message.shape[0]
    src, dst = edge_index[0], edge_index[1]

    def conv(h, W, root, b):
        hW = np.einsum('nf,rfo->nro', h, W)
        m = hW[src, edge_type]
        key_dr = dst * n_rel + edge_type
        cnt = np.zeros(n_nodes * n_rel, h.dtype)
        np.add.at(cnt, key_dr, 1.0)
        nrm = 1.0 / np.maximum(cnt[key_dr], 1.0)
        agg = np.zeros((n_nodes, W.shape[2]), h.dtype)
        np.add.at(agg, dst, m * nrm[:, None])
        return agg + h @ root + b

    h = np.maximum(conv(x, W1, root1, b1), 0)
    node_emb = conv(h, W2, root2, b2)
    disc = embed_table[message[:, 0].astype(np.int32)]
    cont = np.maximum(message[:, 1:2].astype(np.float32) @ cont_w + cont_b, 0)
    mrep = np.maximum(np.concatenate([disc, cont], 1) @ msg_w + msg_b, 0)
    scores = (node_emb * mrep[batch]).sum(1)
    cnts = np.bincount(batch, minlength=n_graphs)
    start = np.cumsum(cnts) - cnts
    pos = np.arange(n_nodes) - start[batch]
    logits = np.full((n_graphs, int(max_nodes)), -np.inf, np.float32)
    ok = pos < int(max_nodes)  # jax .at[].set drops OOB indices; match that
    logits[batch[ok], pos[ok]] = scores.astype(np.float32)[ok]
    return logits


def kernel(**inputs):
    message = np.asarray(inputs["message"], np.float32)
    x = np.asarray(inputs["x"], np.float32)
    edge_index = np.asarray(inputs["edge_index"])
    edge_type = np.asarray(inputs["edge_type"])
    batch = np.asarray(inputs["batch"])
    max_nodes = int(np.asarray(inputs["max_nodes"]))
    W1 = np.asarray(inputs["W1"], np.float32)
    root1 = np.asarray(inputs["root1"], np.float32)
    b1 = np.asarray(inputs["b1"], np.float32)
    W2 = np.asarray(inputs["W2"], np.float32)
    root2 = np.asarray(inputs["root2"], np.float32)
    b2 = np.asarray(inputs["b2"], np.float32)
    embed_table = np.asarray(inputs["embed_table"], np.float32)
    cont_w = np.asarray(inputs["cont_w"], np.float32)
    cont_b = np.asarray(inputs["cont_b"], np.float32)
    msg_w = np.asarray(inputs["msg_w"], np.float32)
    msg_b = np.asarray(inputs["msg_b"], np.float32)

    ng = message.shape[0]
    src, dst = edge_index[0].astype(np.int64), edge_index[1].astype(np.int64)
    et = edge_type.astype(np.int64)

    regular = (
        ng == NG_FULL
        and x.shape == (NG_FULL * NPG, F)
        and max_nodes == NPG
        and W1.shape == (R, F, F)
        and src.shape[0] == NG_FULL * EPG
        and embed_table.shape == (1000, F)
        and np.array_equal(batch, np.repeat(np.arange(ng), NPG))
        and np.array_equal(src // NPG, np.repeat(np.arange(ng), EPG))
        and np.array_equal(dst // NPG, np.repeat(np.arange(ng), EPG))
        and et.min() >= 0 and et.max() < R
        and message[:, 0].min() >= 0 and message[:, 0].max() < 1000
    )
    if not regular:
        return _np_reference(**inputs)

    # ---- host index preprocessing: normalized per-graph operator B ----
    import ml_dtypes
    bf16 = ml_dtypes.bfloat16
    eg = dst // NPG
    dst_l = dst % NPG
    src_l = src % NPG
    key = et * NPG + dst_l                       # relation-major local key
    gk = eg * KEYS + key
    cnt = np.bincount(gk, minlength=NG_FULL * KEYS).astype(np.float32)
    norm = 1.0 / np.maximum(cnt, 1.0)
    B = np.zeros((NG_FULL * NPG, KEYS), np.float32)
    np.add.at(B, (eg * NPG + src_l, key), norm[gk])
    B = B.astype(bf16).reshape(NG_FULL, NPG, KEYS)

    counts = [32, 32, 31, 31, 31, 31, 31, 31]
    starts = np.concatenate([[0], np.cumsum(counts)])[:-1]
    tok = message[:, 0].astype(np.int64)
    contv = message[:, 1]

    # weights (shared across cores)
    shared = {
        "W1L": _bf16(W1.transpose(1, 0, 2).reshape(128, R * 128)),
        "ROOT1": _bf16(root1),
        "B1": b1.reshape(128, 1).astype(np.float32),
        "W2T": _bf16(W2.transpose(2, 0, 1).reshape(128, R * 128)),
        "ROOT2T": _bf16(root2.T),
        "B2": _bf16(b2.reshape(128, 1)),
        "CONTW": _bf16(cont_w),
        "CONTB": cont_b.reshape(128, 1).astype(np.float32),
        "MSGW": _bf16(msg_w.reshape(2, 128, 128).transpose(1, 0, 2).reshape(128, 256)),
        "MSGB": msg_b.reshape(128, 1).astype(np.float32),
    }
    embl = np.zeros((1024, F), np.float32)
    embl[:1000] = embed_table
    shared["EMBL"] = _bf16(embl.reshape(8, 128, F).transpose(1, 0, 2).reshape(128, 8 * F))

    xb = x.astype(bf16)
    in_maps = []
    for c in range(NCORES):
        g0, gc = int(starts[c]), counts[c]
        xg = xb[g0 * NPG:(g0 + gc) * NPG].reshape(gc, NPG, F)
        # XL: [p, (g,c,f)] node-chunked lhsT layout
        xlv = np.zeros((128, G, 2, F), bf16)
        xlv[:, :gc, 0, :] = xg[:, 0:128].transpose(1, 0, 2)
        xlv[:C1, :gc, 1, :] = xg[:, 128:NPG].transpose(1, 0, 2)
        # XT: x^T
        xtv = np.zeros((128, G * NPG), bf16)
        xtv[:, :gc * NPG] = xg.reshape(gc * NPG, F).T
        # B chunks: [128|72 rows, g*KEYS + k]
        Bg = B[g0:g0 + gc]                       # [gc, 200, 1600]
        bc0 = np.zeros((128, G * KEYS), bf16)
        bc0[:, :gc * KEYS] = Bg[:, 0:128].transpose(1, 0, 2).reshape(128, gc * KEYS)
        bc1 = np.zeros((C1, G * KEYS), bf16)
        bc1[:, :gc * KEYS] = Bg[:, 128:NPG].transpose(1, 0, 2).reshape(C1, gc * KEYS)
        # message-side inputs
        selv = np.zeros((1024, G), np.float32)
        selv[tok[g0:g0 + gc], np.arange(gc)] = 1.0
        sel = _bf16(selv.reshape(8, 128, G).transpose(1, 0, 2).reshape(128, 8 * G))
        cont_row = np.zeros((1, G), np.float32)
        cont_row[0, :gc] = contv[g0:g0 + gc]

        m = dict(shared)
        m.update({
            "XL": np.ascontiguousarray(xlv.reshape(128, G * 2 * F)),
            "XT": xtv, "BC0": bc0, "BC1": bc1,
            "SEL": sel, "CONT": _bf16(cont_row),
        })
        in_maps.append(m)

    from concourse.bass_utils import run_bass_kernel_spmd
    if "nc" not in _COMPILED:
        _COMPILED["nc"] = _build_program()
    global _LAST_IN_MAPS
    _LAST_IN_MAPS = in_maps
    res = run_bass_kernel_spmd(_COMPILED["nc"], in_maps, core_ids=list(range(NCORES)))

    out = np.empty((NG_FULL, NPG), np.float32)
    for c in range(NCORES):
        g0, gc = int(starts[c]), counts[c]
        out[g0:g0 + gc] = res.results[c]["OUT"].reshape(G, NPG)[:gc]
    return out
